# revision 1
# baseline (speedup 1.0000x reference)
"""DeepSpeed-style fused MLP (residual-add + LayerNorm + GEMM1 + GELU + GEMM2
+ bias/residual add) on 8 Trainium2 NeuronCores.

Strategy (tensor-parallel over the intermediate dim, DeepSpeed style):
  - Each core LayerNorms its 1/8 of the tokens in fp32, PE-transposes the
    normalized activations to [H, tok] bf16, and one AllGather shares them.
  - attn_nw/attn_nb are folded into inter_w/inter_b on the host (exact).
  - Per 512-token group: GEMM1 (bf16, fp32 accum) -> bias+gelu on ScalarE ->
    GEMM2 -> fp32 partial output -> ReduceScatter over cores (pipelined
    behind the next group's compute) -> owning core adds residual_add +
    output_b and writes its disjoint token slice.
  - Host concatenates the 8 disjoint token slices.

Self-contained: hardcodes the problem shapes (B=2, S=2048, H=4096, I=16384).
"""
import numpy as np
import ml_dtypes

BF16_NP = ml_dtypes.bfloat16

# problem shapes
B, S, H, I = 2, 2048, 4096, 16384
T = B * S
NC = 8
P = 128
NT = 512                 # tokens per group (= one GEMM1 moving-operand width)
G = T // NT              # groups
TPC = T // NC            # tokens LayerNormed per core
ISH = I // NC            # intermediate shard per core
IC = ISH // P            # i-chunks per core
HC = H // P              # h-chunks
FN = 512                 # GEMM2 free width
HN = H // FN             # h-strips in GEMM2
OWN = NT // NC           # tokens owned per core per group (ReduceScatter)
EPS = 1e-12

_BUILD_CACHE = {}


def _build(t, h, i_dim, stage='full', reps=1):
    import concourse.bass as bass
    import concourse.mybir as mybir
    import concourse.tile as tile
    from concourse import bacc
    from concourse.bass import ts
    from concourse.masks import make_identity
    from contextlib import ExitStack

    f32 = mybir.dt.float32
    bf16 = mybir.dt.bfloat16
    A = mybir.ActivationFunctionType
    OP = mybir.AluOpType

    nt = NT
    g_cnt = t // nt
    tpc = t // NC
    ish = i_dim // NC
    ic_cnt = ish // P
    hc_cnt = h // P
    fn = min(FN, h)
    hn_cnt = h // fn
    own = nt // NC
    fch = min(512, h)
    rg = [list(range(NC))]

    nc = bacc.Bacc(trn_type="TRN2", num_devices=NC)

    xin = nc.dram_tensor("xin", (tpc, h), f32, kind="ExternalInput")
    xres = nc.dram_tensor("xres", (tpc, h), f32, kind="ExternalInput")
    ln_bias = nc.dram_tensor("ln_bias", (h,), f32, kind="ExternalInput")
    bias_sum = nc.dram_tensor("bias_sum", (h,), f32, kind="ExternalInput")
    wiT = nc.dram_tensor("wiT", (ic_cnt, P, hc_cnt * P), bf16,
                         kind="ExternalInput")
    bi = nc.dram_tensor("bi", (P, ic_cnt), f32, kind="ExternalInput")
    woT = nc.dram_tensor("woT", (hn_cnt, P, ic_cnt * fn), bf16,
                         kind="ExternalInput")
    fa_in = nc.dram_tensor("fa_in", (g_cnt * own, h), f32,
                           kind="ExternalInput")
    fa_res = nc.dram_tensor("fa_res", (g_cnt * own, h), f32,
                            kind="ExternalInput")
    out_ext = nc.dram_tensor("out", (g_cnt * own, h), f32,
                             kind="ExternalOutput")

    def bcast(ap, parts):
        return bass.AP(tensor=ap.tensor, offset=ap.offset,
                       ap=[[0, parts]] + list(ap.ap))

    with tile.TileContext(nc) as tc, ExitStack() as ctx:
        consts = ctx.enter_context(tc.tile_pool(name="consts", bufs=1))
        dram = ctx.enter_context(tc.tile_pool(name="dram", bufs=1,
                                              space="DRAM"))
        dram2 = ctx.enter_context(tc.tile_pool(name="dram2", bufs=2,
                                               space="DRAM"))

        rep_bsum = consts.tile([P, h], f32)
        nc.sync.dma_start(rep_bsum[:], bcast(bias_sum[:], P))
        bi_sb = consts.tile([P, ic_cnt], f32)
        nc.sync.dma_start(bi_sb[:], bi[:])
        ident = consts.tile([P, P], bf16)
        make_identity(nc, ident[:])
        eps_t = consts.tile([P, 1], f32)
        nc.vector.memset(eps_t[:], EPS)

        for rep in range(reps):
            ag_in = dram.tile([h, tpc], bf16)
            ag_out = dram.tile([NC * h, tpc], bf16, addr_space="Shared")

            # ---- Stage 1: local LayerNorm (+ attn bias) and PE transpose ----
            with tc.tile_pool(name="lnp", bufs=2) as lnp, \
                 tc.tile_pool(name="sqp", bufs=1) as sqp, \
                 tc.tile_pool(name="repp", bufs=1) as repp, \
                 tc.tile_pool(name="pstr", bufs=2, space="PSUM") as pstr, \
                 tc.tile_pool(name="stgp", bufs=4) as stgp:
                rep_bias = repp.tile([P, h], f32)
                nc.sync.dma_start(rep_bias[:], bcast(ln_bias[:], P))
                for tb in range(tpc // P):
                    x_t = lnp.tile([P, h], f32, tag="x_t")
                    nc.sync.dma_start(x_t[:], xin[ts(tb, P)])
                    r_t = lnp.tile([P, h], f32, tag="r_t")
                    nc.sync.dma_start(r_t[:], xres[ts(tb, P)])
                    ra = lnp.tile([P, h], f32, tag="ra")
                    nc.vector.tensor_add(ra[:], x_t[:], r_t[:])
                    nc.vector.tensor_add(ra[:], ra[:], rep_bias[:])
                    # stats
                    nmean = lnp.tile([P, 1], f32, tag="nmean")
                    nc.vector.reduce_sum(nmean[:], ra[:],
                                         axis=mybir.AxisListType.X)
                    nc.scalar.mul(nmean[:], nmean[:], -1.0 / h)
                    sq = sqp.tile([P, h], f32, tag="sq")
                    nc.vector.tensor_mul(sq[:], ra[:], ra[:])
                    ssq = lnp.tile([P, 1], f32, tag="ssq")
                    nc.vector.reduce_sum(ssq[:], sq[:],
                                         axis=mybir.AxisListType.X)
                    # var = ssq/h - mean^2 ; rstd = 1/sqrt(var + eps)
                    var = lnp.tile([P, 1], f32, tag="var")
                    nc.vector.tensor_scalar_mul(var[:], ssq[:], 1.0 / h)
                    msq = lnp.tile([P, 1], f32, tag="msq")
                    nc.vector.tensor_mul(msq[:], nmean[:], nmean[:])
                    nc.vector.tensor_sub(var[:], var[:], msq[:])
                    rstd = lnp.tile([P, 1], f32, tag="rstd")
                    nc.scalar.activation(rstd[:], var[:], A.Sqrt, bias=eps_t[:])
                    nc.vector.reciprocal(rstd[:], rstd[:])
                    # ln = (ra - mean) * rstd, cast to bf16
                    lnb = lnp.tile([P, h], bf16, tag="lnb")
                    nc.vector.tensor_scalar(lnb[:], ra[:], nmean[:], rstd[:],
                                            op0=OP.add, op1=OP.mult)
                    # transpose 128x128 blocks -> ag_in[h, tpc]
                    for hcb in range(hc_cnt):
                        ps_tr = pstr.tile([P, P], bf16, tag="ps_tr")
                        nc.tensor.transpose(ps_tr[:], lnb[:, ts(hcb, P)],
                                            ident[:])
                        stg = stgp.tile([P, P], bf16, tag="stg")
                        nc.vector.tensor_copy(stg[:], ps_tr[:])
                        nc.sync.dma_start(ag_in[ts(hcb, P), ts(tb, P)], stg[:])

            if stage in ('ag', 'full'):
                nc.gpsimd.collective_compute(
                    "AllGather", mybir.AluOpType.bypass, replica_groups=rg,
                    ins=[ag_in[:].opt()], outs=[ag_out[:].opt()])

            # ---- Stage 2: per-group GEMM1 -> gelu -> GEMM2 -> RS -> final ----
            if stage not in ('ln', 'ag'):
                with tc.tile_pool(name="lntp", bufs=2) as lntp, \
                   tc.tile_pool(name="intp", bufs=2) as intp, \
                   tc.tile_pool(name="w1p", bufs=3) as w1p, \
                   tc.tile_pool(name="w2p", bufs=2) as w2p, \
                   tc.tile_pool(name="obp", bufs=3) as obp, \
                   tc.tile_pool(name="fap", bufs=2) as fap, \
                   tc.tile_pool(name="ps1", bufs=3, space="PSUM") as ps1p, \
                   tc.tile_pool(name="ps2", bufs=2, space="PSUM") as ps2p:
                  for g in range(g_cnt):
                      lnT = lntp.tile([P, hc_cnt, nt], bf16, tag="lnT")
                      if stage == 'g1l':
                          nc.vector.memset(lnT[:], 0.0)
                      for j in ([] if stage == 'g1l' else range(nt // tpc)):
                          blk = g * (nt // tpc) + j
                          nc.sync.dma_start(
                              lnT[:, :, ts(j, tpc)],
                              ag_out[blk * h:(blk + 1) * h, :].rearrange(
                                  "(hc p) t -> p hc t", p=P))
                      interT = (None if stage == 'g1nomm' else
                                intp.tile([P, ic_cnt, nt], bf16, tag="interT"))
                      # GEMM1: C1T[i, t] += wiT[h, i].T @ lnT[h, t]
                      w1_cache = None
                      for ic in range(ic_cnt):
                          if stage != 'g1w' or w1_cache is None:
                              w1 = w1p.tile([P, hc_cnt * P], bf16, tag="w1")
                              nc.sync.dma_start(w1[:], wiT[ic])
                              w1_cache = w1
                          w1 = w1_cache if stage == 'g1w' else w1
                          if stage == 'g1nomm':
                              continue
                          ps = ps1p.tile([P, nt], f32, tag="ps")
                          for hcb in range(hc_cnt):
                              nc.tensor.matmul(ps[:], w1[:, ts(hcb, P)],
                                               lnT[:, hcb, :],
                                               start=(hcb == 0),
                                               stop=(hcb == hc_cnt - 1))
                          nc.scalar.activation(interT[:, ic, :], ps[:],
                                               A.Gelu_apprx_tanh,
                                               bias=bi_sb[:, ic:ic + 1])
                      if stage in ('g1', 'g1w', 'g1l', 'g1nomm'):
                          continue
                      # GEMM2: out[t, hblk] += interT[i, t].T @ woT[i, hblk]
                      rs_in = dram2.tile([nt, h], f32, tag="rs_in")
                      for hn in range(hn_cnt):
                          w2 = w2p.tile([P, ic_cnt * fn], bf16, tag="w2")
                          nc.sync.dma_start(w2[:], woT[hn])
                          for tsb in range(nt // P):
                              ps2 = ps2p.tile([P, fn], f32, tag="ps2")
                              for ic in range(ic_cnt):
                                  nc.tensor.matmul(ps2[:],
                                                   interT[:, ic, ts(tsb, P)],
                                                   w2[:, ts(ic, fn)],
                                                   start=(ic == 0),
                                                   stop=(ic == ic_cnt - 1))
                              ob = obp.tile([P, fn], f32, tag="ob")
                              nc.vector.tensor_copy(ob[:], ps2[:])
                              nc.sync.dma_start(rs_in[ts(tsb, P), ts(hn, fn)],
                                                ob[:])
                      if stage == 'nors':
                          continue
                      rs_out = dram2.tile([own, h], f32, tag="rs_out")
                      nc.gpsimd.collective_compute(
                          "ReduceScatter", mybir.AluOpType.add, replica_groups=rg,
                          ins=[rs_in[:].opt()], outs=[rs_out[:].opt()])
                      # final: out = rs_out + (fa_in + fa_res) + (bias + output_b)
                      for ch in range(h // fch):
                          fo = fap.tile([own, fch], f32, tag="fo")
                          nc.sync.dma_start(fo[:], rs_out[:, ts(ch, fch)])
                          fi = fap.tile([own, fch], f32, tag="fi")
                          nc.sync.dma_start(fi[:], fa_in[ts(g, own), ts(ch, fch)])
                          fr = fap.tile([own, fch], f32, tag="fr")
                          nc.sync.dma_start(fr[:], fa_res[ts(g, own), ts(ch, fch)])
                          nc.vector.tensor_add(fo[:], fo[:], fi[:])
                          nc.vector.tensor_add(fo[:], fo[:], fr[:])
                          nc.vector.tensor_add(fo[:], fo[:],
                                               rep_bsum[:own, ts(ch, fch)])
                          nc.sync.dma_start(out_ext[ts(g, own), ts(ch, fch)],
                                            fo[:])
    nc.finalize()
    return nc


def get_nc(t=T, h=H, i_dim=I, stage='full', reps=1):
    key = (t, h, i_dim, stage, reps)
    if key not in _BUILD_CACHE:
        _BUILD_CACHE[key] = _build(t, h, i_dim, stage, reps)
    return _BUILD_CACHE[key]


def prep_in_maps(input, residual, bias, attn_nw, attn_nb, inter_w, inter_b,
                 output_w, output_b, t=T, h=H, i_dim=I):
    nt = NT
    g_cnt = t // nt
    tpc = t // NC
    ish = i_dim // NC
    ic_cnt = ish // P
    hc_cnt = h // P
    fn = min(FN, h)
    hn_cnt = h // fn
    own = nt // NC

    x2 = np.ascontiguousarray(np.asarray(input, dtype=np.float32).reshape(t, h))
    r2 = np.ascontiguousarray(np.asarray(residual, dtype=np.float32).reshape(t, h))
    bias = np.asarray(bias, dtype=np.float32)
    nw = np.asarray(attn_nw, dtype=np.float32)
    nb = np.asarray(attn_nb, dtype=np.float32)
    wi = np.asarray(inter_w, dtype=np.float32)
    ib = np.asarray(inter_b, dtype=np.float32)
    wo = np.asarray(output_w, dtype=np.float32)
    ob = np.asarray(output_b, dtype=np.float32)

    bsum = bias + ob
    x4 = x2.reshape(g_cnt, NC, own, h)
    r4 = r2.reshape(g_cnt, NC, own, h)

    in_maps = []
    for c in range(NC):
        lo, hi = c * ish, (c + 1) * ish
        wi_c = wi[lo:hi]                       # [ish, h]
        wiT_eff = (wi_c * nw[None, :]).T       # [h, ish]
        w1 = np.ascontiguousarray(
            wiT_eff.reshape(hc_cnt, P, ic_cnt, P).transpose(2, 1, 0, 3)
            .reshape(ic_cnt, P, hc_cnt * P)).astype(BF16_NP)
        bi_eff = ib[lo:hi] + nb @ wi_c.T       # [ish]
        bi_c = np.ascontiguousarray(bi_eff.reshape(ic_cnt, P).T)
        woT_c = wo[:, lo:hi].T                 # [ish, h]
        w2 = np.ascontiguousarray(
            woT_c.reshape(ic_cnt, P, hn_cnt, fn).transpose(2, 1, 0, 3)
            .reshape(hn_cnt, P, ic_cnt * fn)).astype(BF16_NP)
        in_maps.append({
            "xin": np.ascontiguousarray(x2[c * tpc:(c + 1) * tpc]),
            "xres": np.ascontiguousarray(r2[c * tpc:(c + 1) * tpc]),
            "ln_bias": bias,
            "bias_sum": bsum,
            "wiT": w1,
            "bi": bi_c,
            "woT": w2,
            "fa_in": np.ascontiguousarray(x4[:, c].reshape(g_cnt * own, h)),
            "fa_res": np.ascontiguousarray(r4[:, c].reshape(g_cnt * own, h)),
        })
    return in_maps


def assemble(results, t=T, h=H):
    g_cnt = t // NT
    own = NT // NC
    out = np.empty((g_cnt, NC, own, h), dtype=np.float32)
    for c in range(NC):
        out[:, c] = results[c]["out"].reshape(g_cnt, own, h)
    return out.reshape(t, h)


def run(inputs, t=T, h=H, i_dim=I, trace=False, stage='full'):
    from concourse import bass_utils
    nc = get_nc(t, h, i_dim, stage)
    in_maps = prep_in_maps(**inputs, t=t, h=h, i_dim=i_dim)
    res = bass_utils.run_bass_kernel_spmd(
        nc, in_maps, core_ids=list(range(NC)), trace=trace)
    out = assemble(res.results, t=t, h=h)
    return out, res


def kernel(**inputs):
    out, _ = run(inputs)
    return out.reshape(B, S, H).astype(np.float32)



# revision 7
# speedup vs baseline: 1.2424x; 1.2424x over previous
"""DeepSpeed-style fused MLP (residual-add + LayerNorm + GEMM1 + GELU + GEMM2
+ bias/residual add) on 8 Trainium2 NeuronCores.

Strategy (data-parallel over tokens — no collectives):
  - Each core owns T/8 = 512 tokens end-to-end. attn_nw/attn_nb are folded
    into inter_w/inter_b on the host; the attention-output bias is folded
    into the input (xin = x + bias), output_b into the final-add residual
    (xresf = residual + output_b).
  - Per core: LayerNorm its 512 tokens in fp32, PE-transpose the normalized
    activations to lnT [H, 512] bf16 (SBUF-resident), GEMM1 (bf16, fp32
    accum) streaming full inter_w -> bias+gelu on ScalarE -> interT
    [I, 512] bf16 (SBUF-resident) -> GEMM2 streaming full output_w ->
    psum + xin + xresf -> write its disjoint token slice.
  - Host concatenates the 8 disjoint token slices.

Self-contained: hardcodes the problem shapes (B=2, S=2048, H=4096, I=16384).
"""
import numpy as np
import ml_dtypes

BF16_NP = ml_dtypes.bfloat16

# problem shapes
B, S, H, I = 2, 2048, 4096, 16384
T = B * S
NC = 8
P = 128
TPC = T // NC            # tokens per core
W2C = 8                  # i-tiles per GEMM2 weight chunk
FN = 512                 # GEMM2 h-strip width
EPS = 1e-12

_BUILD_CACHE = {}


def _build(tpc=TPC, h=H, i_dim=I, reps=1, nd=NC):
    import concourse.mybir as mybir
    import concourse.tile as tile
    from concourse import bacc
    from concourse.bass import ts
    from concourse.masks import make_identity
    from contextlib import ExitStack

    f32 = mybir.dt.float32
    bf16 = mybir.dt.bfloat16
    A = mybir.ActivationFunctionType
    OP = mybir.AluOpType

    hc = h // P
    icnt = i_dim // P
    w2c = min(W2C, icnt)
    icc_cnt = icnt // w2c
    fn = min(FN, h)
    hn_cnt = h // fn
    ntb = tpc // P

    nc = bacc.Bacc(trn_type="TRN2", num_devices=nd)

    # xin = x + attn-output bias (host-folded); xresf = residual + output_b
    xin = nc.dram_tensor("xin", (tpc, h), f32, kind="ExternalInput")
    xres = nc.dram_tensor("xres", (tpc, h), f32, kind="ExternalInput")
    xresf = nc.dram_tensor("xresf", (tpc, h), f32, kind="ExternalInput")
    w1 = nc.dram_tensor("w1", (icnt, P, hc * P), bf16, kind="ExternalInput")
    bi = nc.dram_tensor("bi", (P, icnt), f32, kind="ExternalInput")
    w2 = nc.dram_tensor("w2", (hn_cnt * icc_cnt, P, w2c * fn), bf16,
                        kind="ExternalInput")
    out_ext = nc.dram_tensor("out", (tpc, h), f32, kind="ExternalOutput")

    with tile.TileContext(nc) as tc, ExitStack() as ctx:
        consts = ctx.enter_context(tc.tile_pool(name="consts", bufs=1))
        big = ctx.enter_context(tc.tile_pool(name="big", bufs=1))
        ident = consts.tile([P, P], bf16)
        make_identity(nc, ident[:])
        eps_t = consts.tile([P, 1], f32)
        nc.vector.memset(eps_t[:], EPS)
        bi_sb = consts.tile([P, icnt], f32)
        nc.sync.dma_start(bi_sb[:], bi[:])

        for rep in range(reps):
            lnT = big.tile([P, hc, tpc], bf16, tag="lnT")
            interT = big.tile([P, icnt, tpc], bf16, tag="interT")

            # ---- Stage 1: LayerNorm + PE transpose into lnT [h, tpc] ----
            with tc.tile_pool(name="lnp", bufs=1) as lnp, \
                 tc.tile_pool(name="lnbp", bufs=2) as lnbp, \
                 tc.tile_pool(name="stat", bufs=2) as stat, \
                 tc.tile_pool(name="pstr", bufs=2, space="PSUM") as pstr:
                hh = h // 2
                for tb in range(ntb):
                    ra = lnp.tile([P, h], f32, tag="ra")
                    nc.sync.dma_start(ra[:], xin[ts(tb, P)])
                    lnb = lnbp.tile([P, h], bf16, tag="lnb")
                    for hv in range(2):
                        rh = lnp.tile([P, hh], f32, tag="rh")
                        nc.sync.dma_start(rh[:], xres[ts(tb, P), ts(hv, hh)])
                        nc.vector.tensor_add(ra[:, ts(hv, hh)],
                                             ra[:, ts(hv, hh)], rh[:])
                    nmean = stat.tile([P, 1], f32, tag="nmean")
                    nc.vector.reduce_sum(nmean[:], ra[:],
                                         axis=mybir.AxisListType.X)
                    nc.scalar.mul(nmean[:], nmean[:], -1.0 / h)
                    ssq = stat.tile([P, 1], f32, tag="ssq")
                    # lnb doubles as throwaway Square scratch (overwritten
                    # by the real ln values below)
                    nc.scalar.activation(lnb[:], ra[:], A.Square,
                                         accum_out=ssq[:])
                    var = stat.tile([P, 1], f32, tag="var")
                    nc.vector.tensor_scalar_mul(var[:], ssq[:], 1.0 / h)
                    msq = stat.tile([P, 1], f32, tag="msq")
                    nc.vector.tensor_mul(msq[:], nmean[:], nmean[:])
                    nc.vector.tensor_sub(var[:], var[:], msq[:])
                    rstd = stat.tile([P, 1], f32, tag="rstd")
                    nc.scalar.activation(rstd[:], var[:], A.Sqrt,
                                         bias=eps_t[:])
                    nc.vector.reciprocal(rstd[:], rstd[:])
                    nc.vector.tensor_scalar(lnb[:], ra[:], nmean[:], rstd[:],
                                            op0=OP.add, op1=OP.mult)
                    for hcb in range(hc):
                        ps_tr = pstr.tile([P, P], bf16, tag="ps_tr")
                        nc.tensor.transpose(ps_tr[:], lnb[:, ts(hcb, P)],
                                            ident[:])
                        nc.vector.tensor_copy(lnT[:, hcb, ts(tb, P)], ps_tr[:])

            # ---- Stage 2: GEMM1 -> gelu -> interT; GEMM2 -> final add ----
            with tc.tile_pool(name="w1p", bufs=2) as w1p, \
                 tc.tile_pool(name="w2p", bufs=2) as w2p, \
                 tc.tile_pool(name="fap", bufs=3) as fap, \
                 tc.tile_pool(name="ps1", bufs=2, space="PSUM") as ps1p, \
                 tc.tile_pool(name="ps2", bufs=1, space="PSUM") as ps2p:
                for ic in range(icnt):
                    w1t = w1p.tile([P, hc * P], bf16, tag="w1t")
                    nc.sync.dma_start(w1t[:], w1[ic])
                    ps = ps1p.tile([P, tpc], f32, tag="ps")
                    for hcb in range(hc):
                        nc.tensor.matmul(ps[:], w1t[:, ts(hcb, P)],
                                         lnT[:, hcb, :],
                                         start=(hcb == 0),
                                         stop=(hcb == hc - 1))
                    nc.scalar.activation(interT[:, ic, :], ps[:],
                                         A.Gelu_apprx_tanh,
                                         bias=bi_sb[:, ic:ic + 1])
                for hn in range(hn_cnt):
                    pss = [ps2p.tile([P, fn], f32, tag=f"ps2_{tsb}",
                                     name=f"ps2_{tsb}")
                           for tsb in range(ntb)]
                    for icc in range(icc_cnt):
                        w2t = w2p.tile([P, w2c * fn], bf16, tag="w2t")
                        nc.sync.dma_start(w2t[:], w2[hn * icc_cnt + icc])
                        for tsb in range(ntb):
                            for j in range(w2c):
                                icg = icc * w2c + j
                                nc.tensor.matmul(
                                    pss[tsb][:], interT[:, icg, ts(tsb, P)],
                                    w2t[:, ts(j, fn)],
                                    start=(icc == 0 and j == 0),
                                    stop=(icc == icc_cnt - 1 and
                                          j == w2c - 1))
                    for tsb in range(ntb):
                        fx = fap.tile([P, fn], f32, tag="fx")
                        nc.sync.dma_start(fx[:], xin[ts(tsb, P), ts(hn, fn)])
                        fr = fap.tile([P, fn], f32, tag="fr")
                        nc.sync.dma_start(fr[:], xresf[ts(tsb, P), ts(hn, fn)])
                        nc.vector.tensor_add(fx[:], fx[:], pss[tsb][:])
                        nc.vector.tensor_add(fx[:], fx[:], fr[:])
                        nc.sync.dma_start(out_ext[ts(tsb, P), ts(hn, fn)],
                                          fx[:])
    nc.finalize()
    return nc


def get_nc(tpc=TPC, h=H, i_dim=I, reps=1, nd=NC):
    key = (tpc, h, i_dim, reps, nd)
    if key not in _BUILD_CACHE:
        _BUILD_CACHE[key] = _build(tpc, h, i_dim, reps, nd)
    return _BUILD_CACHE[key]


def prep_weights(bias, attn_nw, attn_nb, inter_w, inter_b, output_w, output_b,
                 h=H, i_dim=I):
    """Host-side weight folding + tiling. Returns dict of shared tensors."""
    nw = np.asarray(attn_nw, dtype=np.float32)
    nb = np.asarray(attn_nb, dtype=np.float32)
    wi = np.asarray(inter_w, dtype=np.float32)
    ib = np.asarray(inter_b, dtype=np.float32)
    wo = np.asarray(output_w, dtype=np.float32)

    hc = h // P
    icnt = i_dim // P
    w2c = min(W2C, icnt)
    icc_cnt = icnt // w2c
    fn = min(FN, h)
    hn_cnt = h // fn

    w_eff = wi * nw[None, :]                     # [I, H]
    w1_host = np.ascontiguousarray(
        w_eff.reshape(icnt, P, hc, P).transpose(0, 3, 2, 1)
        .reshape(icnt, P, hc * P)).astype(BF16_NP)
    bi_eff = ib + wi @ nb                        # [I]
    bi_host = np.ascontiguousarray(bi_eff.reshape(icnt, P).T)
    w2_host = np.ascontiguousarray(
        wo.T.reshape(icc_cnt, w2c, P, hn_cnt, fn).transpose(3, 0, 2, 1, 4)
        .reshape(hn_cnt * icc_cnt, P, w2c * fn)).astype(BF16_NP)
    return {"w1": w1_host, "bi": bi_host, "w2": w2_host}


def prep_in_maps(input, residual, bias, attn_nw, attn_nb, inter_w, inter_b,
                 output_w, output_b):
    x2 = np.asarray(input, dtype=np.float32).reshape(T, H)
    r2 = np.asarray(residual, dtype=np.float32).reshape(T, H)
    bias = np.asarray(bias, dtype=np.float32)
    ob = np.asarray(output_b, dtype=np.float32)

    wts = prep_weights(bias, attn_nw, attn_nb, inter_w, inter_b,
                       output_w, output_b)
    xin_full = x2 + bias[None, :]
    xresf_full = r2 + ob[None, :]

    in_maps = []
    for c in range(NC):
        sl = slice(c * TPC, (c + 1) * TPC)
        in_maps.append({
            "xin": np.ascontiguousarray(xin_full[sl]),
            "xres": np.ascontiguousarray(r2[sl]),
            "xresf": np.ascontiguousarray(xresf_full[sl]),
            **wts,
        })
    return in_maps


def assemble(results):
    return np.concatenate([r["out"] for r in results], axis=0)


def run(inputs, trace=False):
    from concourse import bass_utils
    nc = get_nc()
    in_maps = prep_in_maps(**inputs)
    res = bass_utils.run_bass_kernel_spmd(
        nc, in_maps, core_ids=list(range(NC)), trace=trace)
    return assemble(res.results), res


def kernel(**inputs):
    out, _ = run(inputs)
    return out.reshape(B, S, H).astype(np.float32)
